# revision 13
# baseline (speedup 1.0000x reference)
"""Bass/Trainium2 kernel for nn_BloomEmbedding (hashed embedding lookup).

Strategy (data-parallel over 8 NeuronCores):
  - Replicate the hash tables on every core; shard the 4096x200 input ids
    along the batch axis (512 rows -> 102,400 flat ids per core).
  - On-device per core: compute the 4 xxhash-style 32-bit hashes with
    exact integer arithmetic emulated in 11-bit limbs on the vector
    engine (DVE arithmetic is fp32 internally, exact below 2^24;
    bitwise/shift ops are exact on int32), reduce mod 1e6 via CRT
    (mod 64 x mod 15625), then gather 32-float rows from the four
    [1M, 32] tables with indirect DMAs (the TRN2 dynamic-DMA firmware
    consumes one offset per partition per call, so each call fetches 128
    rows; the per-call Q7 cost grows with the declared source-AP row
    count, so each call passes a 2-row truncated VIEW of its table --
    the firmware computes base + idx*32 without bounds-checking the
    declared shape, which is HW-validated bit-exact) and stream the
    concatenated [id, 128] rows back to DRAM with one large store per
    block.
"""

import numpy as np
from contextlib import ExitStack

import concourse.bass as bass
import concourse.bacc as bacc
import concourse.tile as tile
import concourse.mybir as mybir

TABLE_SIZE = 1_000_000
NUM_HASH = 4
SUB_DIM = 32
EMB_DIM = NUM_HASH * SUB_DIM  # 128
SEED = 42

BATCH = 4096
SEQLEN = 200
N_TOTAL = BATCH * SEQLEN          # 819,200
N_CORES = 8
N_PER_CORE = N_TOTAL // N_CORES   # 102,400

KB = 100                          # ids per partition per block
BLOCK_IDS = 128 * KB              # 12,800 ids per block
N_BLOCKS = N_PER_CORE // BLOCK_IDS  # 8

# Hash multipliers in 11-bit limbs
C1 = 0x7FEB352D
C2 = 0x846CA68B


def _limbs11(v):
    return [v & 2047, (v >> 11) & 2047, (v >> 22) & 1023]


def emit_consts(tc, ctx, kb):
    nc = tc.nc
    i32 = mybir.dt.int32
    W = NUM_HASH * kb
    consts = ctx.enter_context(tc.tile_pool(name="consts", bufs=1))
    seedpat = consts.tile([128, W], i32, name="seedpat")
    htab = consts.tile([128, W], i32, name="htab")
    for h in range(NUM_HASH):
        nc.vector.memset(seedpat[:, h * kb:(h + 1) * kb], SEED + h)
        nc.vector.memset(htab[:, h * kb:(h + 1) * kb], h * TABLE_SIZE)
    return seedpat, htab


def emit_hash_block(tc, hp, idt, seedpat, htab, kb):
    """Emit the 4-seed hash for one block.

    idt: [128, kb] int32 tile of ids. Returns idxt [128, 4*kb] int32 tile
    holding stacked-table row indices, column-interleaved as k*4+h.
    """
    nc = tc.nc
    i32 = mybir.dt.int32
    A = mybir.AluOpType
    W = NUM_HASH * kb
    c1l = _limbs11(C1)
    c2l = _limbs11(C2)

    x = hp.tile([128, W], i32, name="x")
    for h in range(NUM_HASH):
        nc.vector.tensor_copy(x[:, h * kb:(h + 1) * kb], idt[:])

    l0 = hp.tile([128, W], i32, name="l0")
    l1 = hp.tile([128, W], i32, name="l1")
    l2 = hp.tile([128, W], i32, name="l2")
    c = hp.tile([128, W], i32, name="c")
    t = hp.tile([128, W], i32, name="t")
    u = hp.tile([128, W], i32, name="u")
    s1 = hp.tile([128, W], i32, name="s1")
    s2 = hp.tile([128, W], i32, name="s2")
    idxt = hp.tile([128, W], i32, name="idxt")

    def ts(out, in0, sa, sb, op0, op1):
        nc.vector.tensor_scalar(out, in0, sa, sb, op0, op1)

    def tss(out, in0, s, op):
        nc.vector.tensor_single_scalar(out, in0, s, op)

    def tt(out, in0, in1, op):
        nc.vector.tensor_tensor(out, in0, in1, op)

    # ---- x = id + seed, in 11-bit limbs (ids < 2^30, nonnegative) ----
    tss(l0[:], x[:], 2047, A.bitwise_and)
    tt(l0[:], l0[:], seedpat[:], A.add)
    tss(c[:], l0[:], 11, A.logical_shift_right)
    tss(l0[:], l0[:], 2047, A.bitwise_and)
    ts(l1[:], x[:], 11, 2047, A.logical_shift_right, A.bitwise_and)
    tt(l1[:], l1[:], c[:], A.add)
    tss(c[:], l1[:], 11, A.logical_shift_right)
    tss(l1[:], l1[:], 2047, A.bitwise_and)
    tss(l2[:], x[:], 22, A.logical_shift_right)
    tt(l2[:], l2[:], c[:], A.add)

    def xorshift16():
        # y = x >> 16; bit 16 = limb1 bit 5
        ts(t[:], l2[:], 31, 6, A.bitwise_and, A.logical_shift_left)
        tss(u[:], l1[:], 5, A.logical_shift_right)
        tt(t[:], t[:], u[:], A.bitwise_or)
        tt(l0[:], l0[:], t[:], A.bitwise_xor)
        tss(u[:], l2[:], 5, A.logical_shift_right)
        tt(l1[:], l1[:], u[:], A.bitwise_xor)

    def xorshift15():
        # y = x >> 15; bit 15 = limb1 bit 4
        ts(t[:], l2[:], 15, 7, A.bitwise_and, A.logical_shift_left)
        tss(u[:], l1[:], 4, A.logical_shift_right)
        tt(t[:], t[:], u[:], A.bitwise_or)
        tt(l0[:], l0[:], t[:], A.bitwise_xor)
        tss(u[:], l2[:], 4, A.logical_shift_right)
        tt(l1[:], l1[:], u[:], A.bitwise_xor)

    def mult_const(cl):
        # (l2,l1,l0) *= (cl2,cl1,cl0) mod 2^32, 11-bit limbs.
        # All partial products < 2^23; column sums < 2^24 (fp32-exact).
        tss(s1[:], l0[:], cl[1], A.mult)
        tss(t[:], l1[:], cl[0], A.mult)
        tt(s1[:], s1[:], t[:], A.add)
        tss(s2[:], l0[:], cl[2], A.mult)
        tss(t[:], l1[:], cl[1], A.mult)
        tt(s2[:], s2[:], t[:], A.add)
        tss(t[:], l2[:], cl[0], A.mult)
        tt(s2[:], s2[:], t[:], A.add)
        tss(u[:], l0[:], cl[0], A.mult)      # p00
        tss(c[:], u[:], 11, A.logical_shift_right)
        tss(l0[:], u[:], 2047, A.bitwise_and)
        tt(s1[:], s1[:], c[:], A.add)
        tss(c[:], s1[:], 11, A.logical_shift_right)
        tss(l1[:], s1[:], 2047, A.bitwise_and)
        tt(s2[:], s2[:], c[:], A.add)
        tss(l2[:], s2[:], 1023, A.bitwise_and)

    xorshift16()
    mult_const(c1l)
    xorshift15()
    mult_const(c2l)
    xorshift16()

    # ---- idx = x mod 1e6 via CRT(64, 15625) ----
    # a64 = (x mod 64) + 64
    tss(u[:], l0[:], 63, A.bitwise_and)
    tss(u[:], u[:], 64, A.add)
    # y = l0 + l1*2048 + l2*6804  (== x mod 15625 pre-reduction, < 2^24)
    tss(s1[:], l1[:], 2048, A.mult)
    tss(s2[:], l2[:], 6804, A.mult)
    tt(s1[:], s1[:], l0[:], A.add)
    tt(s1[:], s1[:], s2[:], A.add)
    # r = y mod 15625 (reciprocal-mult rounds to int on writeback; the
    # +-1 quotient error is fixed up below)
    tss(c[:], s1[:], float(1.0 / 15625.0), A.mult)
    tss(c[:], c[:], 15625, A.mult)
    tt(s1[:], s1[:], c[:], A.subtract)
    tss(c[:], s1[:], 0, A.is_lt)
    tss(c[:], c[:], 15625, A.mult)
    tt(s1[:], s1[:], c[:], A.add)
    tss(c[:], s1[:], 15624, A.is_gt)
    tss(c[:], c[:], 15625, A.mult)
    tt(s1[:], s1[:], c[:], A.subtract)
    # CRT combine: idx = r + 15625 * ((57*(a - r mod 64)) mod 64)
    tss(t[:], s1[:], 63, A.bitwise_and)
    tt(u[:], u[:], t[:], A.subtract)
    tss(u[:], u[:], 57, A.mult)
    tss(u[:], u[:], 63, A.bitwise_and)
    tss(u[:], u[:], 15625, A.mult)
    tt(s1[:], s1[:], u[:], A.add)
    # + h*table_size (stacked-table row), written interleaved so that
    # the gather's flat index order is (k, h) matching the output row
    # layout [id, hash, 32].
    nc.vector.tensor_copy(
        idxt[:].rearrange("p (k h) -> p h k", h=NUM_HASH), s1[:])
    return idxt


def emit_bloom_kernel(ctx, tc, ids_ap, tab_aps, out_ap, n_ids, kb):
    """ids: [n_ids] i32; tab_aps: 4x [TABLE_SIZE, 32] f32; out: [n_ids, 128]."""
    nc = tc.nc
    i32 = mybir.dt.int32
    f32 = mybir.dt.float32
    n_blocks = n_ids // (128 * kb)
    assert n_ids == n_blocks * 128 * kb

    seedpat, htab = emit_consts(tc, ctx, kb)
    iop = ctx.enter_context(tc.tile_pool(name="io", bufs=2))
    hp = ctx.enter_context(tc.tile_pool(name="hash", bufs=2))
    ep = ctx.enter_context(tc.tile_pool(name="emb", bufs=2))

    ids3 = ids_ap.rearrange("(b p k) -> b p k", b=n_blocks, p=128)
    out3 = out_ap.rearrange("(b p k) d -> b p (k d)", b=n_blocks, p=128)

    for b in range(n_blocks):
        idt = iop.tile([128, kb], i32, name="idt")
        nc.sync.dma_start(idt[:], ids3[b])

        idxt = emit_hash_block(tc, hp, idt, seedpat, htab, kb)

        # HW indirect-DMA semantics: one offset per partition per call,
        # SUB_DIM contiguous elements each. One call per idxt column.
        emb = ep.tile([128, kb * EMB_DIM], f32, name="emb")
        W = NUM_HASH * kb
        for j in range(W):
            nc.gpsimd.indirect_dma_start(
                out=emb[:, j * SUB_DIM:(j + 1) * SUB_DIM],
                out_offset=None,
                in_=tab_aps[j % NUM_HASH][:2],
                in_offset=bass.IndirectOffsetOnAxis(
                    ap=idxt[:, j:j + 1], axis=0),
            )

        nc.scalar.dma_start(out3[b], emb[:])


def emit_bloom_kernel_debug(ctx, tc, ids_ap, idxd_ap, tabs_ap,
                            out_idx_ap, out_emb_ap, kb):
    """Debug: hash-only output + known-index gather output (1 block)."""
    nc = tc.nc
    i32 = mybir.dt.int32
    f32 = mybir.dt.float32
    W = NUM_HASH * kb

    seedpat, htab = emit_consts(tc, ctx, kb)
    iop = ctx.enter_context(tc.tile_pool(name="io", bufs=1))
    hp = ctx.enter_context(tc.tile_pool(name="hash", bufs=1))
    ep = ctx.enter_context(tc.tile_pool(name="emb", bufs=1))

    idt = iop.tile([128, kb], i32, name="idt")
    nc.sync.dma_start(idt[:], ids_ap.rearrange("(p k) -> p k", p=128))
    idxt = emit_hash_block(tc, hp, idt, seedpat, htab, kb)
    nc.scalar.dma_start(out_idx_ap, idxt[:])

    idxd = iop.tile([128, W], i32, name="idxd")
    nc.sync.dma_start(idxd[:], idxd_ap)
    emb = ep.tile([128, kb * EMB_DIM], f32, name="emb")
    nc.gpsimd.indirect_dma_start(
        out=emb[:].rearrange("p (k e) -> p k e", e=SUB_DIM),
        out_offset=None,
        in_=tabs_ap,
        in_offset=bass.IndirectOffsetOnAxis(ap=idxd[:], axis=0),
    )
    nc.scalar.dma_start(out_emb_ap, emb[:])


def build_nc(n_ids=N_PER_CORE, kb=KB, table_size=TABLE_SIZE):
    nc = bacc.Bacc("TRN2", debug=False, num_devices=N_CORES)
    ids = nc.dram_tensor("ids", [n_ids], mybir.dt.int32, kind="ExternalInput")
    tab_aps = [
        nc.dram_tensor(f"tables{h}", [table_size, SUB_DIM], mybir.dt.float32,
                       kind="ExternalInput").ap()
        for h in range(NUM_HASH)
    ]
    out = nc.dram_tensor(
        "out", [n_ids, EMB_DIM], mybir.dt.float32, kind="ExternalOutput")
    with tile.TileContext(nc) as tc:
        with ExitStack() as ctx:
            emit_bloom_kernel(ctx, tc, ids.ap(), tab_aps, out.ap(),
                              n_ids, kb)
    nc.compile()
    return nc


_nc_cache = None


def kernel(input_ids: np.ndarray, tables: np.ndarray) -> np.ndarray:
    global _nc_cache
    from concourse.bass_utils import run_bass_kernel_spmd

    if _nc_cache is None:
        _nc_cache = build_nc()
    nc = _nc_cache

    flat = np.ascontiguousarray(input_ids, dtype=np.int32).reshape(-1)
    shards = flat.reshape(N_CORES, N_PER_CORE)
    tabs4 = np.ascontiguousarray(tables, dtype=np.float32).reshape(
        NUM_HASH, TABLE_SIZE, SUB_DIM)
    in_maps = [
        {"ids": shards[i],
         **{f"tables{h}": tabs4[h] for h in range(NUM_HASH)}}
        for i in range(N_CORES)
    ]
    res = run_bass_kernel_spmd(nc, in_maps, core_ids=list(range(N_CORES)))
    outs = [res.results[i]["out"] for i in range(N_CORES)]
    full = np.concatenate(outs, axis=0)
    return full.reshape(BATCH, SEQLEN, EMB_DIM)
